# revision 8
# baseline (speedup 1.0000x reference)
"""ArcLengthLoss distributed Bass kernel for 8 TRN2 NeuronCores.

Reference computation (see problem spec):
    s = output[:, :, 0]                               # [32, 153]
    A = s[:, a1] - s[:, a2]; a_term = exp(A.mean(1))  # [32]
    b1 = s[:, direct]                                 # [32, NC]
    b2 = sum_l mask(l<seg_len) * s[:, pad_idx[:, l]]  # [32, NC]
    loss = (a_term + |b1-b2|.mean(1)).mean()

Strategy: the per-combo gather/sum is algebraically a matmul against a signed
count matrix:  B[b, c] = sum_k sT[k, b] * W[k, c]  with
W[k, c] = [direct[c] == k] - #{l < seg_len[c] : pad_idx[c, l] == k}.
W is built SPARSELY (18 writes per combo) with the GPSIMD local_scatter
instruction in [combo, section] layout, DMA-transposed to [section, combo],
and contracted on the TensorEngine with the tiny bf16 table.  abs+sum is a
fused VectorE/ScalarE reduction; final scalar assembly happens on the host
from per-core partial vectors (that is the unshard step).

Combos are sharded across the 8 cores (32768 per core after padding).
"""
import sys

if "/opt/trn_rl_repo" not in sys.path:
    sys.path.insert(0, "/opt/trn_rl_repo")

import numpy as np

import concourse.bass as bass  # noqa: F401  (bass types used via bacc/tile)
import concourse.bacc as bacc
import concourse.tile as tile
from concourse import mybir
from concourse.bass_utils import run_bass_kernel_spmd

# ---- problem constants (hardcoded per spec) ----
B = 32            # batch
S = 153           # sections
NA = 136          # a1/a2 pairs
NC = 261972       # combos
L = 17            # max segments per combo
CORES = 8
PERCORE = 32768   # padded combos per core
NTOT = PERCORE * CORES

# ---- kernel tiling ----
P = 128           # partitions (combos per group-column)
G = 8             # groups per scatter tile
TILE = P * G      # combos per tile = 1024
TILES = PERCORE // TILE  # 32
HIBASE = G * 128  # 1024: start of HI region inside a Wt row
WTW = G * 128 + G * 32   # 1280: Wt width (LO 8*128 | HI 8*32)
NI = 18 * G       # scatter indices per partition per tile

_DT = mybir.dt
_CACHE = {}


def build_nc():
    """Build + compile the per-core Bass graph (same graph on all 8 cores)."""
    nc = bacc.Bacc("TRN2", target_bir_lowering=False, debug=False,
                   num_devices=CORES)

    s_d = nc.dram_tensor("output", [B, S, 1], _DT.float32, kind="ExternalInput")
    a1_d = nc.dram_tensor("a1", [NA], _DT.int32, kind="ExternalInput")
    a2_d = nc.dram_tensor("a2", [NA], _DT.int32, kind="ExternalInput")
    dir_d = nc.dram_tensor("direct", [PERCORE], _DT.int32, kind="ExternalInput")
    pad_d = nc.dram_tensor("pad_idx", [PERCORE, L], _DT.int32, kind="ExternalInput")
    seg_d = nc.dram_tensor("seg_len", [PERCORE], _DT.int32, kind="ExternalInput")

    ob_d = nc.dram_tensor("out_b", [128], _DT.float32, kind="ExternalOutput")
    oa_d = nc.dram_tensor("out_a", [B], _DT.float32, kind="ExternalOutput")

    with tile.TileContext(nc) as tc:
        with (
            tc.tile_pool(name="const", bufs=1) as cpool,
            tc.tile_pool(name="inp", bufs=8) as ipool,
            tc.tile_pool(name="mid", bufs=8) as mpool,
            tc.tile_pool(name="wts", bufs=6) as wpool,
            tc.tile_pool(name="acc", bufs=3) as apool,
            tc.tile_pool(name="psum", bufs=6, space="PSUM") as ppool,
            tc.tile_pool(name="psumA", bufs=1, space="PSUM") as papool,
        ):
            # ---- warm up the local_scatter ucode library (~6us IRAM load)
            # concurrently with the first input DMAs
            wdum = cpool.tile([16, 2], _DT.bfloat16)
            idum = cpool.tile([16, 2], _DT.int16)
            ddum = cpool.tile([16, 2], _DT.bfloat16)
            nc.gpsimd.iota(idum[:], pattern=[[1, 2]], base=0, channel_multiplier=0)
            nc.gpsimd.memset(ddum[:], 0.0)
            nc.gpsimd.local_scatter(wdum[:], ddum[:], idum[:],
                                    channels=16, num_elems=2, num_idxs=2)

            # ---- constants
            iota_l = cpool.tile([P, L], _DT.int32)          # 0..16 per row
            nc.gpsimd.iota(iota_l[:], pattern=[[1, L]], base=0,
                           channel_multiplier=0)
            glo = cpool.tile([P, G], _DT.int32)             # g*128
            nc.gpsimd.iota(glo[:], pattern=[[128, G]], base=0,
                           channel_multiplier=0)
            ghi = cpool.tile([P, G], _DT.int32)             # HIBASE-128 + 32g
            nc.gpsimd.iota(ghi[:], pattern=[[32, G]], base=HIBASE - 128,
                           channel_multiplier=0)
            dump68 = cpool.tile([P, G * L], _DT.int16)      # HIBASE+25+32g, x17
            nc.gpsimd.iota(dump68[:], pattern=[[32, G], [0, L]],
                           base=HIBASE + 25, channel_multiplier=0)
            data_c = cpool.tile([P, 18 * G], _DT.bfloat16)  # [+1 x G | -1 x 17G]
            nc.gpsimd.memset(data_c[:], -1.0)
            nc.gpsimd.memset(data_c[:, 0:G], 1.0)

            # ---- table prep: s -> bf16, transposed LO/HI with zero padding
            s_sb = cpool.tile([B, S], _DT.float32)
            nc.scalar.dma_start(s_sb[:], s_d.ap().rearrange("b s o -> b (s o)"))
            s16 = cpool.tile([B, 256], _DT.bfloat16)
            nc.vector.memset(s16[:], 0.0)
            nc.vector.tensor_copy(s16[:, 0:S], s_sb[:])
            sT_lo = cpool.tile([128, B], _DT.bfloat16)
            nc.sync.dma_start_transpose(sT_lo[:], s16[:, 0:128])
            # hi sections replicated 4x along free -> 4 partition-replicas
            s16rep = cpool.tile([B, 128], _DT.bfloat16)
            for r in range(4):
                nc.vector.tensor_copy(s16rep[:, r * 32:(r + 1) * 32],
                                      s16[:, 128:160])
            sT_hi = cpool.tile([128, B], _DT.bfloat16)
            nc.sync.dma_start_transpose(sT_hi[:], s16rep[:])
            # masked variants: rows outside [32g, 32g+32) zeroed, so a full-K
            # matmul against the 4-group-stacked hiT picks out group g only
            sT_hi_g = []
            for g in range(4):
                t = cpool.tile([128, B], _DT.bfloat16, tag=f"sT_hi_{g}")
                nc.vector.memset(t[:], 0.0)
                nc.vector.tensor_copy(t[g * 32:(g + 1) * 32, :],
                                      sT_hi[g * 32:(g + 1) * 32, :])
                sT_hi_g.append(t)

            # ---- A term: one-hot(a1) - one-hot(a2), tiny matmul
            iota_c = cpool.tile([128, 1], _DT.float32)
            nc.gpsimd.iota(iota_c[:], pattern=[[0, 1]], base=0,
                           channel_multiplier=1,
                           allow_small_or_imprecise_dtypes=True)
            iota_ch = cpool.tile([32, 1], _DT.float32)
            nc.gpsimd.iota(iota_ch[:], pattern=[[0, 1]], base=128,
                           channel_multiplier=1,
                           allow_small_or_imprecise_dtypes=True)
            a1b = cpool.tile([128, NA], _DT.int32)
            nc.scalar.dma_start(a1b[:],
                                a1_d.ap().unsqueeze(0).broadcast_to([128, NA]))
            a2b = cpool.tile([128, NA], _DT.int32)
            nc.scalar.dma_start(a2b[:],
                                a2_d.ap().unsqueeze(0).broadcast_to([128, NA]))
            wa_lo = cpool.tile([128, NA], _DT.bfloat16)
            oh2 = cpool.tile([128, NA], _DT.bfloat16)
            nc.vector.tensor_scalar(wa_lo[:], a1b[:], iota_c[:], None,
                                    op0=mybir.AluOpType.is_equal)
            nc.vector.tensor_scalar(oh2[:], a2b[:], iota_c[:], None,
                                    op0=mybir.AluOpType.is_equal)
            nc.vector.tensor_tensor(wa_lo[:], wa_lo[:], oh2[:],
                                    op=mybir.AluOpType.subtract)
            wa_hi = cpool.tile([32, NA], _DT.bfloat16)
            oh2h = cpool.tile([32, NA], _DT.bfloat16)
            nc.vector.tensor_scalar(wa_hi[:], a1b[0:32, :], iota_ch[:], None,
                                    op0=mybir.AluOpType.is_equal)
            nc.vector.tensor_scalar(oh2h[:], a2b[0:32, :], iota_ch[:], None,
                                    op0=mybir.AluOpType.is_equal)
            nc.vector.tensor_tensor(wa_hi[:], wa_hi[:], oh2h[:],
                                    op=mybir.AluOpType.subtract)
            psa = papool.tile([B, NA], _DT.float32, tag="psa")
            nc.tensor.matmul(psa[:], sT_lo[:], wa_lo[:], start=True, stop=False)
            nc.tensor.matmul(psa[:], sT_hi_g[0][0:32, :], wa_hi[:],
                             start=False, stop=True)
            asum = cpool.tile([B, 1], _DT.float32)
            nc.vector.tensor_reduce(asum[:], psa[:], axis=mybir.AxisListType.X,
                                    op=mybir.AluOpType.add)
            nc.scalar.dma_start(oa_d.ap().unsqueeze(1), asum[:])

            # ---- main loop over combo tiles, software-pipelined emission:
            # stages are emitted at tile-skews (deepest first) so each
            # engine's in-order stream interleaves across tiles and DMA /
            # semaphore latencies are hidden.
            # PE column tiling stacks 4 tile-halves at psum partition
            # offsets 0/32/64/96 of one bank: 4x psum depth, 4x fewer and
            # 4x wider reductions.
            bacc_t = cpool.tile([128, TILES // 2], _DT.float32)
            st = [dict() for _ in range(TILES)]
            ps_roll = {}

            def s0_load(t):
                c0 = t * TILE
                d = st[t]
                d["pad"] = ipool.tile([P, G, L], _DT.int32, tag="pad", name=f"pad_{t}")
                nc.scalar.dma_start(
                    d["pad"][:],
                    pad_d.ap()[c0:c0 + TILE, :].rearrange(
                        "(p g) l -> p g l", p=P))
                d["seg"] = ipool.tile([P, G], _DT.int32, tag="seg", name=f"seg_{t}")
                nc.scalar.dma_start(
                    d["seg"][:],
                    seg_d.ap()[c0:c0 + TILE].rearrange("(p g) -> p g", p=P))
                d["dir"] = ipool.tile([P, G], _DT.int32, tag="dir", name=f"dir_{t}")
                nc.scalar.dma_start(
                    d["dir"][:],
                    dir_d.ap()[c0:c0 + TILE].rearrange("(p g) -> p g", p=P))

            def s1_prep(t):
                d = st[t]
                pad_t, seg_t, dir_t = d["pad"], d["seg"], d["dir"]
                m16 = mpool.tile([P, G, L], _DT.int16, tag="m16")
                nc.vector.tensor_tensor(
                    m16[:],
                    iota_l[:].unsqueeze(1).broadcast_to([P, G, L]),
                    seg_t[:].unsqueeze(2).broadcast_to([P, G, L]),
                    op=mybir.AluOpType.is_ge)
                hflag = mpool.tile([P, G, L], _DT.int16, tag="hflag")
                nc.vector.tensor_scalar(hflag[:], pad_t[:], 128, None,
                                        op0=mybir.AluOpType.is_ge)
                idx16 = mpool.tile([P, 18 * G], _DT.int16, tag="idx16")
                idxp = idx16[:, G:].rearrange("p (g l) -> p g l", g=G)
                hi16 = mpool.tile([P, G, L], _DT.int16, tag="hi16")
                nc.vector.tensor_tensor(
                    idxp, pad_t[:],
                    glo[:].unsqueeze(2).broadcast_to([P, G, L]),
                    op=mybir.AluOpType.add)
                nc.vector.tensor_tensor(
                    hi16[:], pad_t[:],
                    ghi[:].unsqueeze(2).broadcast_to([P, G, L]),
                    op=mybir.AluOpType.add)
                nc.vector.copy_predicated(idxp, hflag[:], hi16[:])
                nc.vector.copy_predicated(idxp, m16[:], dump68[:])
                dflag = mpool.tile([P, G], _DT.int16, tag="dflag")
                nc.vector.tensor_scalar(dflag[:], dir_t[:], 128, None,
                                        op0=mybir.AluOpType.is_ge)
                dhi = mpool.tile([P, G], _DT.int16, tag="dhi")
                nc.vector.tensor_tensor(dhi[:], dir_t[:], ghi[:],
                                        op=mybir.AluOpType.add)
                nc.vector.tensor_tensor(idx16[:, 0:G], dir_t[:], glo[:],
                                        op=mybir.AluOpType.add)
                nc.vector.copy_predicated(idx16[:, 0:G], dflag[:], dhi[:])
                d["idx16"] = idx16

            def s2_scatter(t):
                d = st[t]
                wt = wpool.tile([P, WTW], _DT.bfloat16, tag="wt")
                nc.gpsimd.local_scatter(wt[:], data_c[:], d["idx16"][:],
                                        channels=P, num_elems=WTW, num_idxs=NI)
                d["wt"] = wt

            def s3_transpose(t):
                d = st[t]
                wt = d["wt"]
                # one batched block-transpose per region
                # (3D out AP -> out[s, g, p] = wt[p, g*128 + s])
                wT_lo = wpool.tile([128, G * 128], _DT.bfloat16, tag="wTlo")
                nc.sync.dma_start_transpose(
                    wT_lo[:].rearrange("s (g p) -> s g p", g=G),
                    wt[:, 0:HIBASE])
                hiT = wpool.tile([128, 256], _DT.bfloat16, tag="hiT")
                nc.sync.dma_start_transpose(
                    hiT[:].rearrange("r (b p) -> r b p", b=2),
                    wt[:, HIBASE:WTW])
                d["wT_lo"], d["hiT"] = wT_lo, hiT

            def s4_matmul(t):
                d = st[t]
                if t % 2 == 0:
                    ps_roll["ps"] = ppool.tile([128, 512], _DT.float32,
                                               tag="psB4", name=f"psB4_{t}")
                psum = ps_roll["ps"]
                d["psum"] = psum
                for j in range(2):
                    a = (2 * t + j) % 4
                    sub = psum[32 * a:32 * (a + 1), :]
                    nc.tensor.matmul(
                        sub, sT_lo[:],
                        d["wT_lo"][:, j * 512:(j + 1) * 512],
                        start=True, stop=False, skip_group_check=True,
                        tile_position=(0, 32 * a))
                    for gg in range(4):
                        nc.tensor.matmul(
                            sub[:, gg * 128:(gg + 1) * 128], sT_hi_g[gg][:],
                            d["hiT"][:, j * 128:(j + 1) * 128],
                            start=False, stop=True, skip_group_check=True,
                            tile_position=(0, 32 * a))

            def s5_reduce(t):
                if t % 2 == 1:
                    psum = st[t]["psum"]
                    col = t // 2
                    if col % 2 == 0:
                        nc.vector.tensor_reduce(
                            bacc_t[:, col:col + 1], psum[:],
                            axis=mybir.AxisListType.X,
                            op=mybir.AluOpType.add, apply_absolute_value=True)
                    else:
                        trash = apool.tile([128, 512], _DT.bfloat16,
                                           tag="trash", name=f"trash_{t}")
                        nc.scalar.activation(
                            trash[:], psum[:],
                            mybir.ActivationFunctionType.Abs,
                            accum_out=bacc_t[:, col:col + 1])
                st[t] = None  # release references

            stages = [s0_load, s1_prep, s2_scatter, s3_transpose, s4_matmul,
                      s5_reduce]
            NS = len(stages)
            for step in range(TILES + NS - 1):
                # deepest stage first within a step
                for si in reversed(range(NS)):
                    t = step - si
                    if 0 <= t < TILES:
                        stages[si](t)

            bsum = cpool.tile([128, 1], _DT.float32)
            nc.vector.tensor_reduce(bsum[:], bacc_t[:],
                                    axis=mybir.AxisListType.X,
                                    op=mybir.AluOpType.add)
            nc.scalar.dma_start(ob_d.ap().unsqueeze(1), bsum[:])

    nc.compile()
    return nc


def _host_fixup(direct, pad, seg):
    """Detect combos whose scatter targets collide (duplicate active pad
    sections, or direct == an active pad section).  Those cannot be expressed
    by the 0/±1 scatter; neutralize them on-device and return their row
    indices so the host computes their contribution exactly.  Zero rows for
    the reference tables (all active sections of a combo are distinct there).
    """
    n = direct.shape[0]
    lane = np.arange(L)[None, :]
    act = np.where(lane < seg[:, None], pad, 2000 + lane)  # distinct sentinels
    d_eff = np.where(direct < S, direct, 3000)
    t = np.concatenate([d_eff[:, None], act], axis=1)
    t.sort(axis=1)
    dup = (t[:, 1:] == t[:, :-1]).any(axis=1)
    return np.nonzero(dup)[0]


def prepare(inputs):
    """Shard + fix up inputs.  Returns (in_maps, host_abs)."""
    s = np.asarray(inputs["output"], dtype=np.float32)
    a1 = np.asarray(inputs["a1"], dtype=np.int32)
    a2 = np.asarray(inputs["a2"], dtype=np.int32)
    direct = np.asarray(inputs["direct"], dtype=np.int32).copy()
    pad = np.asarray(inputs["pad_idx"], dtype=np.int32).copy()
    seg = np.asarray(inputs["seg_len"], dtype=np.int32).copy()

    # general-correctness fallback for collision rows (none for the
    # reference tables)
    host_abs = 0.0
    bad = _host_fixup(direct, pad, seg)
    if bad.size:
        sv = s[:, :, 0]
        for c in bad:
            m = (np.arange(L) < seg[c]).astype(np.float32)
            b2 = (sv[:, pad[c]] * m[None, :]).sum(axis=1)
            host_abs += float(np.abs(sv[:, direct[c]] - b2).sum())
        direct[bad] = S       # -> dump slot, contributes 0 on device
        seg[bad] = 0

    # pad to NTOT with neutral rows
    npad = NTOT - direct.shape[0]
    direct_p = np.concatenate([direct, np.full(npad, S, np.int32)])
    pad_p = np.concatenate([pad, np.zeros((npad, L), np.int32)])
    seg_p = np.concatenate([seg, np.zeros(npad, np.int32)])

    in_maps = []
    for i in range(CORES):
        sl = slice(i * PERCORE, (i + 1) * PERCORE)
        in_maps.append({
            "output": s, "a1": a1, "a2": a2,
            "direct": np.ascontiguousarray(direct_p[sl]),
            "pad_idx": np.ascontiguousarray(pad_p[sl]),
            "seg_len": np.ascontiguousarray(seg_p[sl]),
        })
    return in_maps, host_abs


def combine(outs, host_abs):
    total_abs = host_abs + sum(float(outs[i]["out_b"].sum())
                               for i in range(CORES))
    mean_a = float(np.exp(outs[0]["out_a"] / NA).mean())
    val = mean_a + total_abs / (B * NC)
    return np.asarray(val, dtype=np.float32)


def get_nc():
    if "nc" not in _CACHE:
        _CACHE["nc"] = build_nc()
    return _CACHE["nc"]


def kernel(**inputs) -> np.ndarray:
    in_maps, host_abs = prepare(inputs)
    res = run_bass_kernel_spmd(get_nc(), in_maps, core_ids=list(range(CORES)))
    return combine(res.results, host_abs)
